# revision 28
# baseline (speedup 1.0000x reference)
"""Trainium2 Bass kernel for one training-mode timestep of a 2-layer CLAPP RSNN.

Reference computation (B=32, NIN=NH=1024, f32):
  per layer:  cur = x @ W.T
              mem' = 0.95*mem + cur ; spk = (mem' > 1) ; mem'' = mem' - spk
              tr'  = 0.95*tr + x
              loss_b = -bf * sum_o spk[b,o] * (prev[b,o] - mean_o prev[b,:])
              dW[b] = bf * outer(prev[b]*surrogate(mem''[b]-1), tr'[b])
  outputs: spk1, mem0'', mem1'', stack([loss0.mean(), loss1.mean()]), dW0, dW1

Sharding: data-parallel over batch, 4 samples per core on 8 NeuronCores;
weights replicated.  bf (+-1 scalar) is folded into prev_spk on the host
(dW and the loss are linear in prev_spk; spikes/mem do not depend on it).

Per-core dataflow (all f32):
  - W0/W1 row-blocks DMA'd in, transposed on the TensorEngine ([128,128]
    tiles via identity matmul) into WT = W.T laid out [i_part, (i_chunk, h)].
  - states are processed in "T layout" [neuron_part, sample]: cur.T chunks
    accumulate in PSUM from WT tiles (stationary) x x.T (moving).
  - traces are broadcast row->128 partitions with a ones[1,128] matmul; the
    dW outer product is then per-partition-scalar multiplies on DVE/ACT into
    [128, 4*1024] staging tiles, one per 128-row chunk of dW, DMA'd out as a
    single strided 2 MiB transfer each.
  - losses are PE reductions (spk.T as stationary x [prev.T | ones]) giving
    [4,5] per layer; host finishes -(s1 - mean(prev')*s2) and averages.
"""

import sys

sys.path.insert(0, "/opt/trn_rl_repo")

from contextlib import ExitStack

import numpy as np

import concourse.bass as bass
import concourse.tile as tile
from concourse import bacc
from concourse import mybir
from concourse.masks import make_identity

N_CORES = 8
B = 32
BL = B // N_CORES  # samples per core
N = 1024  # NIN == NH
P = 128
C = N // P  # 8 chunks
BETA = 0.95
PI = float(np.pi)
PI3 = PI**3
F32 = mybir.dt.float32
AL = mybir.AluOpType
AF = mybir.ActivationFunctionType

# Each [4,1024] row-layout tensor lives in its own tile at base partition 0
# (PE matmul requires operand base partitions in {0,32,64} and equal for
# lhsT/rhs; compute engines want operands partition-aligned).
ROW_NAMES = ("inp", "tr0", "tr1", "mem0", "mem1", "ps0", "ps1",
             "tr0p", "tr1p", "spk0r", "mem0r", "mem1r", "spk1r")


def _emit(nc, tc, ctx, io, n_iters=1, hw_loop=0):
    sp = ctx.enter_context(tc.tile_pool(name="small", bufs=1))
    wraw = ctx.enter_context(tc.tile_pool(name="wraw", bufs=4))
    wtp = ctx.enter_context(tc.tile_pool(name="wtp", bufs=3))
    bcpool = ctx.enter_context(tc.tile_pool(name="bcpool", bufs=8))
    stg = ctx.enter_context(tc.tile_pool(name="stg", bufs=4))
    nwk = ctx.enter_context(tc.tile_pool(name="nwk", bufs=2))
    pp = ctx.enter_context(tc.tile_pool(name="pp", bufs=1, space="PSUM"))
    pcur = ctx.enter_context(tc.tile_pool(name="pcur", bufs=2, space="PSUM"))
    pwt = ctx.enter_context(tc.tile_pool(name="pwt", bufs=2, space="PSUM"))

    # persistent tiles
    rows = {nm: sp.tile([4, N], F32, name=f"r_{nm}") for nm in ROW_NAMES}
    rows["loss"] = sp.tile([4, 10], F32, name="r_loss")
    ident = sp.tile([128, 128], F32)
    emask = sp.tile([4, BL * 128], F32)  # emask[k, b*128+m] = (k == b)
    negone = sp.tile([128, 1], F32)
    inpT = sp.tile([128, BL * C], F32)
    ps05 = sp.tile([128, 5 * C], F32)
    ps15 = sp.tile([128, 5 * C], F32)
    mem0T = sp.tile([128, BL * C], F32)
    mem1T = sp.tile([128, BL * C], F32)
    spk0T = sp.tile([128, BL * C], F32)
    spk1T = sp.tile([128, BL * C], F32)
    m0pT = sp.tile([128, BL * C], F32)
    m1pT = sp.tile([128, BL * C], F32)
    a0T = sp.tile([128, BL * C], F32)
    a1T = sp.tile([128, BL * C], F32)

    make_identity(nc, ident)
    # emask[k, b*128+m] = (k == b): zero, then fill 1.0 where (k - b) == 0
    nc.gpsimd.memset(emask, 0.0)
    nc.gpsimd.affine_select(
        out=emask.rearrange("k (b m) -> k b m", m=128),
        in_=emask.rearrange("k (b m) -> k b m", m=128),
        compare_op=AL.not_equal,
        fill=1.0,
        base=0,
        pattern=[[-1, BL], [0, 128]],
        channel_multiplier=1,
    )
    nc.gpsimd.memset(negone, -1.0)

    t = dict(rows=rows, ident=ident, emask=emask, negone=negone, inpT=inpT,
             ps05=ps05, ps15=ps15, mem0T=mem0T, mem1T=mem1T,
             spk0T=spk0T, spk1T=spk1T, m0pT=m0pT, m1pT=m1pT,
             a0T=a0T, a1T=a1T)
    pools = dict(wraw=wraw, wtp=wtp, bcpool=bcpool, stg=stg, nwk=nwk,
                 pp=pp, pcur=pcur, pwt=pwt)
    if hw_loop:
        # hardware loop for timing: body traced once, looped on device
        with tc.For_i(0, hw_loop, 1, hint_engines=(mybir.EngineType.PE,)):
            _emit_iter(nc, io, t, pools)
    else:
        for _ in range(n_iters):
            _emit_iter(nc, io, t, pools)


def _emit_iter(nc, io, t, pools):
    rows, ident, emask = t["rows"], t["ident"], t["emask"]
    wraw, bcpool, stg, nwk = pools["wraw"], pools["bcpool"], pools["stg"], pools["nwk"]
    wtp = pools["wtp"]
    pp, pcur, pwt = pools["pp"], pools["pcur"], pools["pwt"]

    # ---- input DMAs ----
    for name in ("inp", "tr0", "tr1", "mem0", "mem1", "ps0", "ps1"):
        nc.sync.dma_start(out=rows[name], in_=io[name])
    w0b = []
    for l in range(C):
        wblk = wraw.tile([128, N], F32, tag="wblk")
        nc.sync.dma_start(out=wblk, in_=io["W0"][l * P:(l + 1) * P, :])
        w0b.append(wblk)
    w1b = []
    for l in range(C):
        wblk = wraw.tile([128, N], F32, tag="wblk")
        nc.sync.dma_start(out=wblk, in_=io["W1"][l * P:(l + 1) * P, :])
        w1b.append(wblk)

    # ---- transpose [4,1024] row tensors into T layout [128, C*4] ----
    def pack_T(dst, src):
        ppk = pp.tile([128, BL * C], F32, tag="ppk")
        for c in range(C):
            nc.tensor.transpose(
                ppk[:, c * BL:(c + 1) * BL],
                src[0:4, c * P:(c + 1) * P],
                ident[0:4, 0:4],
            )
        nc.scalar.copy(dst, ppk)

    def pack_T5(dst, src):
        # like pack_T but into [128, C*5] with column 4 of each group = 1.0
        ppk = pp.tile([128, BL * C], F32, tag="ppk")
        for c in range(C):
            nc.tensor.transpose(
                ppk[:, c * BL:(c + 1) * BL],
                src[0:4, c * P:(c + 1) * P],
                ident[0:4, 0:4],
            )
        dv = dst.rearrange("p (c f) -> p c f", f=5)
        nc.scalar.copy(dv[:, :, 0:4], ppk.rearrange("p (c b) -> p c b", b=BL))
        nc.gpsimd.memset(dv[:, :, 4:5], 1.0)

    pack_T(t["inpT"], rows["inp"])
    pack_T(t["mem0T"], rows["mem0"])
    pack_T(t["mem1T"], rows["mem1"])
    pack_T5(t["ps05"], rows["ps0"])
    pack_T5(t["ps15"], rows["ps1"])

    # ---- trace update + broadcast rows to 128 partitions ----
    # broadcast of row b via PE: lhsT = emask[:, b*128:(b+1)*128] (one-hot
    # column selector, K=4), rhs = the 4 trace rows -> out[m,n] = tr[b,n].
    def bcast4(src):
        out = []
        for b in range(BL):
            bc = bcpool.tile([128, N], F32, tag="bc")
            for s in range(2):
                pbc = pp.tile([128, 512], F32, tag="pbc")
                nc.tensor.matmul(
                    pbc, lhsT=emask[:, b * 128:(b + 1) * 128],
                    rhs=src[0:4, s * 512:(s + 1) * 512],
                    start=True, stop=True,
                )
                nc.scalar.copy(bc[:, s * 512:(s + 1) * 512], pbc)
            out.append(bc)
        return out

    nc.vector.scalar_tensor_tensor(
        out=rows["tr0p"], in0=rows["tr0"],
        scalar=BETA, in1=rows["inp"], op0=AL.mult, op1=AL.add,
    )
    bc0 = bcast4(rows["tr0p"])

    # ---- one layer ----
    def layer(wblocks, rhsT, memT, ps5, spkT, mpT, aT, bc, o_dW):
        lossp = pp.tile([4, 5], F32, tag="ploss")
        for l in range(C):
            # transpose W row-block l -> wtl[:, c, :] = W[l-block, c-block].T
            # (these 8 tiles are exactly the lhsT set for cur chunk l)
            wtl = wtp.tile([128, C * P], F32, tag="wtl")
            wtlv = wtl.rearrange("p (c h) -> p c h", h=P)
            for g in range(2):
                pw = pwt.tile([128, 512], F32, tag="pwt")
                for q in range(4):
                    c = g * 4 + q
                    nc.tensor.transpose(
                        pw[:, q * P:(q + 1) * P],
                        wblocks[l][:, c * P:(c + 1) * P], ident,
                    )
                cp = nc.vector.tensor_copy if g == 0 else nc.scalar.copy
                cp(
                    wtlv[:, g * 4:(g + 1) * 4, :],
                    pw.rearrange("p (q h) -> p q h", h=P),
                )
            # cur.T chunk l accumulates over the 8 contraction chunks
            cps = pcur.tile([128, BL], F32, tag="cur")
            for c in range(C):
                nc.tensor.matmul(
                    cps, lhsT=wtlv[:, c, :],
                    rhs=rhsT[:, c * BL:(c + 1) * BL],
                    start=(c == 0), stop=(c == C - 1),
                )
            # neuron state + surrogate for chunk l
            sl = slice(l * BL, (l + 1) * BL)
            memp = nwk.tile([128, BL], F32, tag="memp")
            nc.vector.scalar_tensor_tensor(
                out=memp, in0=memT[:, sl], scalar=BETA, in1=cps,
                op0=AL.mult, op1=AL.add,
            )
            nc.vector.tensor_scalar(spkT[:, sl], memp, 1.0, None, op0=AL.is_gt)
            nc.vector.tensor_sub(mpT[:, sl], memp, spkT[:, sl])
            xm1 = nwk.tile([128, BL], F32, tag="xm1")
            nc.vector.tensor_scalar(xm1, mpT[:, sl], -1.0, None, op0=AL.add)
            sq = nwk.tile([128, BL], F32, tag="sq")
            nc.vector.tensor_mul(sq, xm1, xm1)
            den = nwk.tile([128, BL], F32, tag="den")
            nc.vector.tensor_scalar(den, sq, PI3, PI, op0=AL.mult, op1=AL.add)
            rec = nwk.tile([128, BL], F32, tag="rec")
            nc.vector.reciprocal(rec, den)
            ps5v = ps5.rearrange("p (c f) -> p c f", f=5)
            nc.vector.tensor_mul(aT[:, sl], ps5v[:, l, 0:4], rec)
            # loss reduction: [4,5] += spk_chunk.T @ [prev' | 1]
            nc.tensor.matmul(
                lossp, lhsT=spkT[:, sl], rhs=ps5[:, l * 5:(l + 1) * 5],
                start=(l == 0), stop=(l == C - 1),
            )
            # dW rows for chunk l: stage[p, b*N + j] = a[l*128+p, b] * tr'[b, j]
            stage = stg.tile([128, BL * N], F32, tag="stage")
            for b in range(BL):
                scal = aT[:, l * BL + b:l * BL + b + 1]
                dst = stage[:, b * N:(b + 1) * N]
                if b == 2:
                    nc.scalar.activation(dst, bc[b], AF.Copy, bias=0.0, scale=scal)
                elif b == 3:
                    nc.gpsimd.tensor_scalar_mul(dst, bc[b], scal)
                else:
                    nc.vector.tensor_scalar_mul(dst, bc[b], scal)
            # alternate the two HWDGE rings so dW stores drain on both
            eng = nc.scalar if l % 2 == 0 else nc.sync
            eng.dma_start(
                out=o_dW[:, l * P:(l + 1) * P, :].rearrange("b p j -> p b j"),
                in_=stage.rearrange("p (b j) -> p b j", b=BL),
            )
        return lossp

    lossp0 = layer(w0b, t["inpT"], t["mem0T"], t["ps05"],
                   t["spk0T"], t["m0pT"], t["a0T"], bc0, io["o_dW0"])
    nc.vector.tensor_copy(rows["loss"][0:4, 0:5], lossp0)

    # ---- spk0 back to row layout, layer-1 trace + broadcast ----
    def to_rows(srcT, dst):
        for g in range(2):
            pr = pp.tile([4, 512], F32, tag="prow")
            for q in range(4):
                c = g * 4 + q
                nc.tensor.transpose(
                    pr[0:4, q * P:(q + 1) * P],
                    srcT[:, c * BL:(c + 1) * BL], ident,
                )
            nc.vector.tensor_copy(
                dst[0:4, g * 512:(g + 1) * 512], pr[0:4, :]
            )

    to_rows(t["spk0T"], rows["spk0r"])
    nc.vector.scalar_tensor_tensor(
        out=rows["tr1p"], in0=rows["tr1"],
        scalar=BETA, in1=rows["spk0r"], op0=AL.mult, op1=AL.add,
    )
    bc1 = bcast4(rows["tr1p"])

    lossp1 = layer(w1b, t["spk0T"], t["mem1T"], t["ps15"],
                   t["spk1T"], t["m1pT"], t["a1T"], bc1, io["o_dW1"])
    nc.vector.tensor_copy(rows["loss"][0:4, 5:10], lossp1)

    # ---- outputs ----
    to_rows(t["m0pT"], rows["mem0r"])
    to_rows(t["m1pT"], rows["mem1r"])
    to_rows(t["spk1T"], rows["spk1r"])
    nc.sync.dma_start(out=io["o_mem0"], in_=rows["mem0r"])
    nc.sync.dma_start(out=io["o_mem1"], in_=rows["mem1r"])
    nc.sync.dma_start(out=io["o_spk1"], in_=rows["spk1r"])
    nc.sync.dma_start(
        out=io["o_loss"].rearrange("k b f -> b k f"),
        in_=rows["loss"].rearrange("b (k f) -> b k f", k=2),
    )


def build(n_iters=1, hw_loop=0):
    nc = bacc.Bacc("TRN2", debug=False, num_devices=N_CORES)
    io = {}
    for name, shape in (
        ("inp", [BL, N]), ("W0", [N, N]), ("W1", [N, N]),
        ("mem0", [BL, N]), ("mem1", [BL, N]),
        ("ps0", [BL, N]), ("ps1", [BL, N]),
        ("tr0", [BL, N]), ("tr1", [BL, N]),
    ):
        io[name] = nc.dram_tensor(name, shape, F32, kind="ExternalInput").ap()
    for name, shape in (
        ("o_spk1", [BL, N]), ("o_mem0", [BL, N]), ("o_mem1", [BL, N]),
        ("o_loss", [2, BL, 5]),
        ("o_dW0", [BL, N, N]), ("o_dW1", [BL, N, N]),
    ):
        io[name] = nc.dram_tensor(name, shape, F32, kind="ExternalOutput").ap()
    with tile.TileContext(nc) as tc:
        with ExitStack() as ctx:
            _emit(nc, tc, ctx, io, n_iters=n_iters, hw_loop=hw_loop)
    nc.compile()
    return nc


class Runner:
    """Compile once, execute many times (replicates bass2jax.run_bass_via_pjrt
    multi-core path but keeps the jitted callable across calls)."""

    def __init__(self, nc, n_cores=N_CORES):
        import jax
        from concourse import bass2jax

        bass2jax.install_neuronx_cc_hook()
        self.nc = nc
        self.n_cores = n_cores
        self.jax = jax
        partition_name = (
            nc.partition_id_tensor.name if nc.partition_id_tensor else None
        )
        in_names, out_names, out_avals = [], [], []
        self.zero_shapes = []
        for alloc in nc.m.functions[0].allocations:
            if not isinstance(alloc, mybir.MemoryLocationSet):
                continue
            name = alloc.memorylocations[0].name
            if alloc.kind == "ExternalInput":
                if name != partition_name:
                    in_names.append(name)
            elif alloc.kind == "ExternalOutput":
                out_names.append(name)
                shape = tuple(alloc.tensor_shape)
                dtype = mybir.dt.np(alloc.dtype)
                out_avals.append(jax.core.ShapedArray(shape, dtype))
                self.zero_shapes.append((shape, dtype))
        self.n_params = len(in_names)
        self.out_names = list(out_names)
        all_in = in_names + out_names
        if partition_name is not None:
            all_in.append(partition_name)
        self.in_names = all_in
        self.out_avals = out_avals

        from jax.sharding import Mesh, PartitionSpec, NamedSharding

        try:
            from jax.experimental.shard_map import shard_map
        except ImportError:
            from jax.shard_map import shard_map

        def _body(*args):
            operands = list(args)
            if partition_name is not None:
                operands.append(bass2jax.partition_id_tensor())
            outs = bass2jax._bass_exec_p.bind(
                *operands,
                out_avals=tuple(out_avals),
                in_names=tuple(all_in),
                out_names=tuple(out_names),
                lowering_input_output_aliases=(),
                sim_require_finite=True,
                sim_require_nnan=True,
                nc=nc,
            )
            return tuple(outs)

        devices = jax.devices()[:n_cores]
        assert len(devices) == n_cores
        self.mesh = Mesh(np.asarray(devices), ("core",))
        n_outs = len(out_names)
        donate = tuple(range(self.n_params, self.n_params + n_outs))
        in_specs = (PartitionSpec("core"),) * (self.n_params + n_outs)
        out_specs = (PartitionSpec("core"),) * n_outs
        self.sharding = NamedSharding(self.mesh, PartitionSpec("core"))
        self.fn = jax.jit(
            shard_map(
                _body, mesh=self.mesh, in_specs=in_specs,
                out_specs=out_specs, check_rep=False,
            ),
            donate_argnums=donate, keep_unused=True,
        )
        self._zeros_fn = jax.jit(
            lambda: tuple(
                self.jax.numpy.zeros((n_cores * s[0], *s[1:]), d)
                for s, d in self.zero_shapes
            ),
            out_shardings=tuple([self.sharding] * n_outs),
        )

    def put_inputs(self, in_maps):
        """Concat per-core input dicts and device_put once."""
        concat = [
            np.concatenate(
                [np.asarray(m[name]) for m in in_maps], axis=0
            )
            for name in self.in_names[: self.n_params]
        ]
        return [
            self.jax.device_put(a, self.sharding) for a in concat
        ]

    def run(self, dev_inputs):
        zeros = self._zeros_fn()
        outs = self.fn(*dev_inputs, *zeros)
        self.jax.block_until_ready(outs)
        return outs

    def results(self, outs):
        res = []
        for c in range(self.n_cores):
            d = {}
            for i, name in enumerate(self.out_names):
                full = np.asarray(outs[i])
                d[name] = full.reshape(
                    self.n_cores, *self.out_avals[i].shape
                )[c]
            res.append(d)
        return res


_RUNNER = None


def _get_runner():
    global _RUNNER
    if _RUNNER is None:
        _RUNNER = Runner(build(n_iters=1))
    return _RUNNER


def kernel(inp, W0, W1, mem0, mem1, prev_spk0, prev_spk1,
           inp_trace0, inp_trace1, bf):
    f = lambda x: np.asarray(x, dtype=np.float32)
    inp, W0, W1 = f(inp), f(W0), f(W1)
    mem0, mem1 = f(mem0), f(mem1)
    ps0 = f(prev_spk0) * float(bf)  # bf folded into prev_spk (linear)
    ps1 = f(prev_spk1) * float(bf)
    tr0, tr1 = f(inp_trace0), f(inp_trace1)

    runner = _get_runner()
    in_maps = []
    for c in range(N_CORES):
        s = slice(c * BL, (c + 1) * BL)
        in_maps.append({
            "inp": inp[s], "W0": W0, "W1": W1,
            "mem0": mem0[s], "mem1": mem1[s],
            "ps0": ps0[s], "ps1": ps1[s],
            "tr0": tr0[s], "tr1": tr1[s],
        })
    outs = runner.run(runner.put_inputs(in_maps))
    res = runner.results(outs)

    spk1 = np.concatenate([r["o_spk1"] for r in res], axis=0)
    mem0o = np.concatenate([r["o_mem0"] for r in res], axis=0)
    mem1o = np.concatenate([r["o_mem1"] for r in res], axis=0)
    dW0 = np.concatenate([r["o_dW0"] for r in res], axis=0)
    dW1 = np.concatenate([r["o_dW1"] for r in res], axis=0)

    # losses: device gives, per layer, s1[b,b'] = sum_o spk[o,b] prev'[o,b']
    # (cols 0..3) and s2[b] = sum_o spk[o,b] (col 4).
    # loss_b = -(s1[b,b] - mean(prev'[b]) * s2[b]);  bf already in prev'.
    l0, l1 = [], []
    for c, r in enumerate(res):
        s = slice(c * BL, (c + 1) * BL)
        mu0 = ps0[s].mean(axis=1)
        mu1 = ps1[s].mean(axis=1)
        lr = r["o_loss"]
        for b in range(BL):
            l0.append(-(lr[0, b, b] - mu0[b] * lr[0, b, 4]))
            l1.append(-(lr[1, b, b] - mu1[b] * lr[1, b, 4]))
    losses = np.array([np.mean(l0), np.mean(l1)], dtype=np.float32)

    return spk1, mem0o, mem1o, losses, dW0, dW1


# revision 37
# speedup vs baseline: 2.0344x; 2.0344x over previous
"""Trainium2 Bass kernel for one training-mode timestep of a 2-layer CLAPP RSNN.

Reference computation (B=32, NIN=NH=1024, f32):
  per layer:  cur = x @ W.T
              mem' = 0.95*mem + cur ; spk = (mem' > 1) ; mem'' = mem' - spk
              tr'  = 0.95*tr + x
              loss_b = -bf * sum_o spk[b,o] * (prev[b,o] - mean_o prev[b,:])
              dW[b] = bf * outer(prev[b]*surrogate(mem''[b]-1), tr'[b])
  outputs: spk1, mem0'', mem1'', stack([loss0.mean(), loss1.mean()]), dW0, dW1

Sharding: data-parallel over batch, 4 samples per core on 8 NeuronCores;
weights replicated.  bf (+-1 scalar) is folded into prev_spk on the host
(dW and the loss are linear in prev_spk; spikes/mem do not depend on it).

Per-core dataflow (all f32):
  - W0/W1 row-blocks DMA'd in, transposed on the TensorEngine ([128,128]
    tiles via identity matmul) into WT = W.T laid out [i_part, (i_chunk, h)].
  - states are processed in "T layout" [neuron_part, sample]: cur.T chunks
    accumulate in PSUM from WT tiles (stationary) x x.T (moving).
  - traces are broadcast row->128 partitions with a ones[1,128] matmul; the
    dW outer product is then per-partition-scalar multiplies on DVE/ACT into
    [128, 4*1024] staging tiles, one per 128-row chunk of dW, DMA'd out as a
    single strided 2 MiB transfer each.
  - losses are PE reductions (spk.T as stationary x [prev.T | ones]) giving
    [4,5] per layer; host finishes -(s1 - mean(prev')*s2) and averages.
"""

import sys

sys.path.insert(0, "/opt/trn_rl_repo")

from contextlib import ExitStack

import numpy as np

import concourse.bass as bass
import concourse.tile as tile
from concourse import bacc
from concourse import mybir
from concourse.masks import make_identity

N_CORES = 8
B = 32
BL = B // N_CORES  # samples per core
N = 1024  # NIN == NH
P = 128
C = N // P  # 8 chunks
BETA = 0.95
PI = float(np.pi)
PI3 = PI**3
F32 = mybir.dt.float32
AL = mybir.AluOpType
AF = mybir.ActivationFunctionType

# bench-only knob (test harness): "full" | "dmaonly" | "dmaonly_bl" | "store_bl"
_BENCH_VARIANT = "full"

# Each [4,1024] row-layout tensor lives in its own tile at base partition 0
# (PE matmul requires operand base partitions in {0,32,64} and equal for
# lhsT/rhs; compute engines want operands partition-aligned).
ROW_NAMES = ("inp", "tr0", "tr1", "mem0", "mem1", "ps0", "ps1",
             "tr0p", "tr1p", "spk0r", "mem0r", "mem1r", "spk1r")


def _emit(nc, tc, ctx, io, n_iters=1, hw_loop=0):
    sp = ctx.enter_context(tc.tile_pool(name="small", bufs=1))
    wraw = ctx.enter_context(tc.tile_pool(name="wraw", bufs=4))
    wtp = ctx.enter_context(tc.tile_pool(name="wtp", bufs=3))
    bcpool = ctx.enter_context(tc.tile_pool(name="bcpool", bufs=8))
    stg = ctx.enter_context(tc.tile_pool(name="stg", bufs=4))
    nwk = ctx.enter_context(tc.tile_pool(name="nwk", bufs=2))
    pp = ctx.enter_context(tc.tile_pool(name="pp", bufs=1, space="PSUM"))
    pcur = ctx.enter_context(tc.tile_pool(name="pcur", bufs=2, space="PSUM"))
    pwt = ctx.enter_context(tc.tile_pool(name="pwt", bufs=3, space="PSUM"))

    # persistent tiles
    rows = {nm: sp.tile([4, N], F32, name=f"r_{nm}") for nm in ROW_NAMES}
    rows["loss"] = sp.tile([4, 10], F32, name="r_loss")
    ident = sp.tile([128, 128], F32)
    inpT = sp.tile([128, BL * C], F32)
    ps05 = sp.tile([128, 5 * C], F32)
    ps15 = sp.tile([128, 5 * C], F32)
    mem0T = sp.tile([128, BL * C], F32)
    mem1T = sp.tile([128, BL * C], F32)
    spk0T = sp.tile([128, BL * C], F32)
    spk1T = sp.tile([128, BL * C], F32)
    m0pT = sp.tile([128, BL * C], F32)
    m1pT = sp.tile([128, BL * C], F32)
    a0T = sp.tile([128, BL * C], F32)
    a1T = sp.tile([128, BL * C], F32)
    cur0T = sp.tile([128, BL * C], F32)
    cur1T = sp.tile([128, BL * C], F32)
    tr0rep = sp.tile([128, N], F32)
    tr1rep = sp.tile([128, N], F32)

    make_identity(nc, ident)
    # quadrant rows 4..31 of the replicated-trace tiles are never written by
    # the per-iteration [4-quadrant, 4-row] DMA; zero them once so
    # stream_shuffle's full-tile read sees initialized data
    nc.gpsimd.memset(tr0rep, 0.0)
    nc.gpsimd.memset(tr1rep, 0.0)

    t = dict(rows=rows, ident=ident, inpT=inpT,
             ps05=ps05, ps15=ps15, mem0T=mem0T, mem1T=mem1T,
             spk0T=spk0T, spk1T=spk1T, m0pT=m0pT, m1pT=m1pT,
             a0T=a0T, a1T=a1T, cur0T=cur0T, cur1T=cur1T,
             tr0rep=tr0rep, tr1rep=tr1rep)
    pools = dict(wraw=wraw, wtp=wtp, bcpool=bcpool, stg=stg, nwk=nwk,
                 pp=pp, pcur=pcur, pwt=pwt)
    if hw_loop:
        # hardware loop for timing: body traced once, looped on device
        with tc.For_i(0, hw_loop, 1, hint_engines=(mybir.EngineType.PE,)):
            _emit_iter(nc, io, t, pools)
    else:
        for _ in range(n_iters):
            _emit_iter(nc, io, t, pools)


def _emit_iter(nc, io, t, pools):
    rows, ident = t["rows"], t["ident"]
    wraw, bcpool, stg, nwk = pools["wraw"], pools["bcpool"], pools["stg"], pools["nwk"]
    wtp = pools["wtp"]
    pp, pcur, pwt = pools["pp"], pools["pcur"], pools["pwt"]

    # ---- input DMAs (W0 row-blocks first: they gate the compute lead-in) ----
    if _BENCH_VARIANT.startswith("dmaonly"):
        # DMA-skeleton bench: loads + garbage dW stores, nothing else
        for o_dW in (io["o_dW0"], io["o_dW1"]):
            for l in range(C):
                wblk = wraw.tile([128, N], F32, tag="wblk", name="wblk")
                nc.sync.dma_start(out=wblk, in_=io["W0"][l * P:(l + 1) * P, :])
                stage = stg.tile([128, BL * N], F32, tag="stage", name="stage")
                nc.gpsimd.memset(stage[:, 0:4], 0.0)
                eng = nc.scalar if l % 2 == 0 else nc.sync
                if _BENCH_VARIANT == "dmaonly":
                    eng.dma_start(
                        out=o_dW[:, l * P:(l + 1) * P, :].rearrange("b p j -> p b j"),
                        in_=stage.rearrange("p (b j) -> p b j", b=BL),
                    )
                else:
                    for b in range(BL):
                        eng = nc.scalar if (l * BL + b) % 2 == 0 else nc.sync
                        eng.dma_start(
                            out=o_dW[b, l * P:(l + 1) * P, :],
                            in_=stage[:, b * N:(b + 1) * N],
                        )
        return
    w0b = []
    for l in range(C):
        wblk = wraw.tile([128, N], F32, tag="wblk")
        eng = nc.sync if l % 2 == 0 else nc.scalar
        eng.dma_start(out=wblk, in_=io["W0"][l * P:(l + 1) * P, :])
        w0b.append(wblk)
    for name in ("inp", "tr0", "tr1", "mem0", "mem1", "ps0", "ps1"):
        nc.sync.dma_start(out=rows[name], in_=io[name])
    w1b = []
    for l in range(C):
        wblk = wraw.tile([128, N], F32, tag="wblk")
        eng = nc.sync if l % 2 == 0 else nc.scalar
        eng.dma_start(out=wblk, in_=io["W1"][l * P:(l + 1) * P, :])
        w1b.append(wblk)

    # ---- transpose [4,1024] row tensors into T layout [128, C*4] ----
    def pack_T(dst, src):
        ppk = pp.tile([128, BL * C], F32, tag="ppk")
        for c in range(C):
            nc.tensor.transpose(
                ppk[:, c * BL:(c + 1) * BL],
                src[0:4, c * P:(c + 1) * P],
                ident[0:4, 0:4],
            )
        nc.scalar.copy(dst, ppk)

    def pack_T5(dst, src):
        # like pack_T but into [128, C*5] with column 4 of each group = 1.0
        ppk = pp.tile([128, BL * C], F32, tag="ppk")
        for c in range(C):
            nc.tensor.transpose(
                ppk[:, c * BL:(c + 1) * BL],
                src[0:4, c * P:(c + 1) * P],
                ident[0:4, 0:4],
            )
        dv = dst.rearrange("p (c f) -> p c f", f=5)
        nc.scalar.copy(dv[:, :, 0:4], ppk.rearrange("p (c b) -> p c b", b=BL))
        nc.gpsimd.memset(dv[:, :, 4:5], 1.0)

    pack_T(t["inpT"], rows["inp"])
    pack_T(t["mem0T"], rows["mem0"])
    pack_T(t["mem1T"], rows["mem1"])
    pack_T5(t["ps05"], rows["ps0"])
    pack_T5(t["ps15"], rows["ps1"])

    # ---- trace update + broadcast rows to 128 partitions ----
    # the [4,N] trace rows go to a DRAM scratch, come back replicated into
    # rows 0-3 of each 32-partition quadrant (4-quadrant strided read), and a
    # DVE stream_shuffle (mask=[b]*32, per-quadrant) broadcasts row b to all
    # 128 partitions -- no TensorE/PSUM involvement.
    def bcast4(src_rows, scratch, rep):
        nc.sync.dma_start(out=scratch, in_=src_rows)
        for q in range(4):
            nc.scalar.dma_start(out=rep[q * 32:q * 32 + 4, :], in_=scratch)
        out = []
        for b in range(BL):
            bc = bcpool.tile([128, N], F32, tag="bc")
            nc.vector.stream_shuffle(bc, rep, mask=[b] * 32)
            out.append(bc)
        return out

    nc.vector.scalar_tensor_tensor(
        out=rows["tr0p"], in0=rows["tr0"],
        scalar=BETA, in1=rows["inp"], op0=AL.mult, op1=AL.add,
    )
    bc0 = bcast4(rows["tr0p"], io["tr0p_d"], t["tr0rep"])

    # ---- one layer ----
    # Phase A (per chunk l): W row-block transposes + 8 cur matmuls + PSUM->
    # SBUF copy.  Phase B (once): one batched DVE neuron chain on the whole
    # [128, 32] state.  Phase C: loss matmuls + 64 fills / 64 stores stream.
    # Batching phase B removes ~100 cross-engine handoffs from the critical
    # path; the stores overlap the next layer's phase A.
    def layer(wblocks, rhsT, memT, ps5, spkT, mpT, aT, bc, o_dW, curT):
        lossp = pp.tile([4, 5], F32, tag="ploss")
        for half in range(2):
            _layer_half(wblocks, rhsT, memT, ps5, spkT, mpT, aT, bc, o_dW,
                        curT, lossp, half)
        return lossp

    def _layer_half(wblocks, rhsT, memT, ps5, spkT, mpT, aT, bc, o_dW,
                    curT, lossp, half):
        hc = C // 2
        for l in range(half * hc, (half + 1) * hc):
            # transpose W row-block l -> wtl[:, c, :] = W[l-block, c-block].T
            # (these 8 tiles are exactly the lhsT set for cur chunk l)
            wtl = wtp.tile([128, C * P], F32, tag="wtl")
            wtlv = wtl.rearrange("p (c h) -> p c h", h=P)
            for g in range(2):
                pw = pwt.tile([128, 512], F32, tag="pwt")
                for q in range(4):
                    c = g * 4 + q
                    nc.tensor.transpose(
                        pw[:, q * P:(q + 1) * P],
                        wblocks[l][:, c * P:(c + 1) * P], ident,
                    )
                cp = nc.vector.tensor_copy if g == 0 else nc.scalar.copy
                cp(
                    wtlv[:, g * 4:(g + 1) * 4, :],
                    pw.rearrange("p (q h) -> p q h", h=P),
                )
            # cur.T chunk l accumulates over the 8 contraction chunks
            cps = pcur.tile([128, BL], F32, tag="cur")
            for c in range(C):
                nc.tensor.matmul(
                    cps, lhsT=wtlv[:, c, :],
                    rhs=rhsT[:, c * BL:(c + 1) * BL],
                    start=(c == 0), stop=(c == C - 1),
                )
            sl = slice(l * BL, (l + 1) * BL)
            nc.scalar.copy(curT[:, sl], cps)
        # batched neuron state + surrogate on this half: [128, hc*BL]
        hs = slice(half * hc * BL, (half + 1) * hc * BL)
        memp = nwk.tile([128, BL * hc], F32, tag="memp")
        nc.vector.scalar_tensor_tensor(
            out=memp, in0=memT[:, hs], scalar=BETA, in1=curT[:, hs],
            op0=AL.mult, op1=AL.add,
        )
        nc.vector.tensor_scalar(spkT[:, hs], memp, 1.0, None, op0=AL.is_gt)
        nc.vector.tensor_sub(mpT[:, hs], memp, spkT[:, hs])
        xm1 = nwk.tile([128, BL * hc], F32, tag="xm1")
        nc.vector.tensor_scalar(xm1, mpT[:, hs], -1.0, None, op0=AL.add)
        sq = nwk.tile([128, BL * hc], F32, tag="sq")
        nc.vector.tensor_mul(sq, xm1, xm1)
        den = nwk.tile([128, BL * hc], F32, tag="den")
        nc.vector.tensor_scalar(den, sq, PI3, PI, op0=AL.mult, op1=AL.add)
        rec = nwk.tile([128, BL * hc], F32, tag="rec")
        nc.vector.reciprocal(rec, den)
        ps5v = ps5.rearrange("p (c f) -> p c f", f=5)
        nc.vector.tensor_mul(
            aT.rearrange("p (c b) -> p c b", b=BL)[:, half * hc:(half + 1) * hc, :],
            ps5v[:, half * hc:(half + 1) * hc, 0:4],
            rec.rearrange("p (c b) -> p c b", b=BL),
        )
        # loss reduction: [4,5] += spk_chunk.T @ [prev' | 1] per chunk
        for l in range(half * hc, (half + 1) * hc):
            nc.tensor.matmul(
                lossp, lhsT=spkT[:, l * BL:(l + 1) * BL],
                rhs=ps5[:, l * 5:(l + 1) * 5],
                start=(l == 0), stop=(l == C - 1),
            )
        # dW: stage[p, b*N + j] = a[l*128+p, b] * tr'[b, j]; contiguous
        # per-sample 512 KB stores alternating the two HWDGE rings
        for l in range(half * hc, (half + 1) * hc):
            stage = stg.tile([128, BL * N], F32, tag="stage")
            for b in range(BL):
                scal = aT[:, l * BL + b:l * BL + b + 1]
                dst = stage[:, b * N:(b + 1) * N]
                if b == 3:
                    nc.scalar.activation(dst, bc[b], AF.Copy, bias=0.0, scale=scal)
                else:
                    nc.vector.tensor_scalar_mul(dst, bc[b], scal)
            for b in range(BL):
                eng = nc.scalar if (l * BL + b) % 2 == 0 else nc.sync
                eng.dma_start(
                    out=o_dW[b, l * P:(l + 1) * P, :],
                    in_=stage[:, b * N:(b + 1) * N],
                )

    lossp0 = layer(w0b, t["inpT"], t["mem0T"], t["ps05"],
                   t["spk0T"], t["m0pT"], t["a0T"], bc0, io["o_dW0"], t["cur0T"])
    nc.vector.tensor_copy(rows["loss"][0:4, 0:5], lossp0)

    # ---- spk0 back to row layout, layer-1 trace + broadcast ----
    def to_rows(srcT, dst):
        for g in range(2):
            pr = pp.tile([4, 512], F32, tag="prow")
            for q in range(4):
                c = g * 4 + q
                nc.tensor.transpose(
                    pr[0:4, q * P:(q + 1) * P],
                    srcT[:, c * BL:(c + 1) * BL], ident,
                )
            nc.vector.tensor_copy(
                dst[0:4, g * 512:(g + 1) * 512], pr[0:4, :]
            )

    to_rows(t["spk0T"], rows["spk0r"])
    nc.vector.scalar_tensor_tensor(
        out=rows["tr1p"], in0=rows["tr1"],
        scalar=BETA, in1=rows["spk0r"], op0=AL.mult, op1=AL.add,
    )
    bc1 = bcast4(rows["tr1p"], io["tr1p_d"], t["tr1rep"])

    lossp1 = layer(w1b, t["spk0T"], t["mem1T"], t["ps15"],
                   t["spk1T"], t["m1pT"], t["a1T"], bc1, io["o_dW1"], t["cur1T"])
    nc.vector.tensor_copy(rows["loss"][0:4, 5:10], lossp1)

    # ---- outputs ----
    to_rows(t["m0pT"], rows["mem0r"])
    to_rows(t["m1pT"], rows["mem1r"])
    to_rows(t["spk1T"], rows["spk1r"])
    nc.sync.dma_start(out=io["o_mem0"], in_=rows["mem0r"])
    nc.sync.dma_start(out=io["o_mem1"], in_=rows["mem1r"])
    nc.sync.dma_start(out=io["o_spk1"], in_=rows["spk1r"])
    nc.sync.dma_start(
        out=io["o_loss"].rearrange("k b f -> b k f"),
        in_=rows["loss"].rearrange("b (k f) -> b k f", k=2),
    )


def build(n_iters=1, hw_loop=0):
    nc = bacc.Bacc("TRN2", debug=False, num_devices=N_CORES)
    io = {}
    for name, shape in (
        ("inp", [BL, N]), ("W0", [N, N]), ("W1", [N, N]),
        ("mem0", [BL, N]), ("mem1", [BL, N]),
        ("ps0", [BL, N]), ("ps1", [BL, N]),
        ("tr0", [BL, N]), ("tr1", [BL, N]),
    ):
        io[name] = nc.dram_tensor(name, shape, F32, kind="ExternalInput").ap()
    for name, shape in (
        ("o_spk1", [BL, N]), ("o_mem0", [BL, N]), ("o_mem1", [BL, N]),
        ("o_loss", [2, BL, 5]),
        ("o_dW0", [BL, N, N]), ("o_dW1", [BL, N, N]),
    ):
        io[name] = nc.dram_tensor(name, shape, F32, kind="ExternalOutput").ap()
    for name in ("tr0p_d", "tr1p_d"):
        io[name] = nc.dram_tensor(name, [BL, N], F32, kind="Internal").ap()
    with tile.TileContext(nc) as tc:
        with ExitStack() as ctx:
            _emit(nc, tc, ctx, io, n_iters=n_iters, hw_loop=hw_loop)
    nc.compile()
    return nc


class Runner:
    """Compile once, execute many times (replicates bass2jax.run_bass_via_pjrt
    multi-core path but keeps the jitted callable across calls)."""

    def __init__(self, nc, n_cores=N_CORES):
        import jax
        from concourse import bass2jax

        bass2jax.install_neuronx_cc_hook()
        self.nc = nc
        self.n_cores = n_cores
        self.jax = jax
        partition_name = (
            nc.partition_id_tensor.name if nc.partition_id_tensor else None
        )
        in_names, out_names, out_avals = [], [], []
        self.zero_shapes = []
        for alloc in nc.m.functions[0].allocations:
            if not isinstance(alloc, mybir.MemoryLocationSet):
                continue
            name = alloc.memorylocations[0].name
            if alloc.kind == "ExternalInput":
                if name != partition_name:
                    in_names.append(name)
            elif alloc.kind == "ExternalOutput":
                out_names.append(name)
                shape = tuple(alloc.tensor_shape)
                dtype = mybir.dt.np(alloc.dtype)
                out_avals.append(jax.core.ShapedArray(shape, dtype))
                self.zero_shapes.append((shape, dtype))
        self.n_params = len(in_names)
        self.out_names = list(out_names)
        all_in = in_names + out_names
        if partition_name is not None:
            all_in.append(partition_name)
        self.in_names = all_in
        self.out_avals = out_avals

        from jax.sharding import Mesh, PartitionSpec, NamedSharding

        try:
            from jax.experimental.shard_map import shard_map
        except ImportError:
            from jax.shard_map import shard_map

        def _body(*args):
            operands = list(args)
            if partition_name is not None:
                operands.append(bass2jax.partition_id_tensor())
            outs = bass2jax._bass_exec_p.bind(
                *operands,
                out_avals=tuple(out_avals),
                in_names=tuple(all_in),
                out_names=tuple(out_names),
                lowering_input_output_aliases=(),
                sim_require_finite=True,
                sim_require_nnan=True,
                nc=nc,
            )
            return tuple(outs)

        devices = jax.devices()[:n_cores]
        assert len(devices) == n_cores
        self.mesh = Mesh(np.asarray(devices), ("core",))
        n_outs = len(out_names)
        donate = tuple(range(self.n_params, self.n_params + n_outs))
        in_specs = (PartitionSpec("core"),) * (self.n_params + n_outs)
        out_specs = (PartitionSpec("core"),) * n_outs
        self.sharding = NamedSharding(self.mesh, PartitionSpec("core"))
        self.fn = jax.jit(
            shard_map(
                _body, mesh=self.mesh, in_specs=in_specs,
                out_specs=out_specs, check_rep=False,
            ),
            donate_argnums=donate, keep_unused=True,
        )
        self._zeros_fn = jax.jit(
            lambda: tuple(
                self.jax.numpy.zeros((n_cores * s[0], *s[1:]), d)
                for s, d in self.zero_shapes
            ),
            out_shardings=tuple([self.sharding] * n_outs),
        )

    def put_inputs(self, in_maps):
        """Concat per-core input dicts and device_put once."""
        concat = [
            np.concatenate(
                [np.asarray(m[name]) for m in in_maps], axis=0
            )
            for name in self.in_names[: self.n_params]
        ]
        return [
            self.jax.device_put(a, self.sharding) for a in concat
        ]

    def run(self, dev_inputs):
        zeros = self._zeros_fn()
        outs = self.fn(*dev_inputs, *zeros)
        self.jax.block_until_ready(outs)
        return outs

    def results(self, outs):
        res = []
        for c in range(self.n_cores):
            d = {}
            for i, name in enumerate(self.out_names):
                full = np.asarray(outs[i])
                d[name] = full.reshape(
                    self.n_cores, *self.out_avals[i].shape
                )[c]
            res.append(d)
        return res


_RUNNER = None


def _get_runner():
    global _RUNNER
    if _RUNNER is None:
        _RUNNER = Runner(build(n_iters=1))
    return _RUNNER


def kernel(inp, W0, W1, mem0, mem1, prev_spk0, prev_spk1,
           inp_trace0, inp_trace1, bf):
    f = lambda x: np.asarray(x, dtype=np.float32)
    inp, W0, W1 = f(inp), f(W0), f(W1)
    mem0, mem1 = f(mem0), f(mem1)
    ps0 = f(prev_spk0) * float(bf)  # bf folded into prev_spk (linear)
    ps1 = f(prev_spk1) * float(bf)
    tr0, tr1 = f(inp_trace0), f(inp_trace1)

    runner = _get_runner()
    in_maps = []
    for c in range(N_CORES):
        s = slice(c * BL, (c + 1) * BL)
        in_maps.append({
            "inp": inp[s], "W0": W0, "W1": W1,
            "mem0": mem0[s], "mem1": mem1[s],
            "ps0": ps0[s], "ps1": ps1[s],
            "tr0": tr0[s], "tr1": tr1[s],
        })
    outs = runner.run(runner.put_inputs(in_maps))
    res = runner.results(outs)

    spk1 = np.concatenate([r["o_spk1"] for r in res], axis=0)
    mem0o = np.concatenate([r["o_mem0"] for r in res], axis=0)
    mem1o = np.concatenate([r["o_mem1"] for r in res], axis=0)
    dW0 = np.concatenate([r["o_dW0"] for r in res], axis=0)
    dW1 = np.concatenate([r["o_dW1"] for r in res], axis=0)

    # losses: device gives, per layer, s1[b,b'] = sum_o spk[o,b] prev'[o,b']
    # (cols 0..3) and s2[b] = sum_o spk[o,b] (col 4).
    # loss_b = -(s1[b,b] - mean(prev'[b]) * s2[b]);  bf already in prev'.
    l0, l1 = [], []
    for c, r in enumerate(res):
        s = slice(c * BL, (c + 1) * BL)
        mu0 = ps0[s].mean(axis=1)
        mu1 = ps1[s].mean(axis=1)
        lr = r["o_loss"]
        for b in range(BL):
            l0.append(-(lr[0, b, b] - mu0[b] * lr[0, b, 4]))
            l1.append(-(lr[1, b, b] - mu1[b] * lr[1, b, 4]))
    losses = np.array([np.mean(l0), np.mean(l1)], dtype=np.float32)

    return spk1, mem0o, mem1o, losses, dW0, dW1


# revision 40
# speedup vs baseline: 2.2802x; 1.1208x over previous
"""Trainium2 Bass kernel for one training-mode timestep of a 2-layer CLAPP RSNN.

Reference computation (B=32, NIN=NH=1024, f32):
  per layer:  cur = x @ W.T
              mem' = 0.95*mem + cur ; spk = (mem' > 1) ; mem'' = mem' - spk
              tr'  = 0.95*tr + x
              loss_b = -bf * sum_o spk[b,o] * (prev[b,o] - mean_o prev[b,:])
              dW[b] = bf * outer(prev[b]*surrogate(mem''[b]-1), tr'[b])
  outputs: spk1, mem0'', mem1'', stack([loss0.mean(), loss1.mean()]), dW0, dW1

Sharding: data-parallel over batch, 4 samples per core on 8 NeuronCores;
weights replicated.  Host folds two pure-input quantities: bf (+-1 scalar)
into prev_spk (dW and the loss are linear in it; spikes/mem don't depend on
it), and the layer-0 trace update tr0' = beta*tr0 + inp (so the kernel
broadcast-reads tr0' straight from DRAM).

Per-core dataflow (all f32), tuned for engine-phase batching (cross-engine
handoffs on TRN2 cost far more than the cost model claims):
  - W0/W1 row-blocks stream in on both HWDGE rings and are transposed on the
    TensorEngine ([128,128] tiles via identity matmul); the 8 transposed
    tiles of row-block l are exactly the lhsT set for cur chunk l, so they
    live in a small cycling pool, never a full W.T.
  - cur.T chunks accumulate in PSUM (WT stationary x x.T moving) and are
    copied to SBUF; per layer-half ONE batched 8-op DVE chain computes
    mem/spike/surrogate for all neurons at once.
  - trace rows are broadcast to 128 partitions by DVE stream_shuffle fed by
    a 4-quadrant replicated DMA read (layer 0: directly from the tr0' input;
    layer 1: via a 16 KB DRAM scratch roundtrip routed through SWDGE so it
    is not queued behind the dW store FIFO on the HWDGE rings).
  - dW outer products: per-partition-scalar multiplies (DVE 3:1 ACT) into
    [128, 4*1024] staging tiles, stored as contiguous per-sample 512 KB
    DMAs alternating both HWDGE rings.
  - losses are PE reductions (spk.T stationary x [prev.T | ones]) giving
    [4,5] per layer; host finishes -(s1 - mean(prev')*s2) and averages.
"""

import sys

sys.path.insert(0, "/opt/trn_rl_repo")

from contextlib import ExitStack

import numpy as np

import concourse.bass as bass
import concourse.tile as tile
from concourse import bacc
from concourse import mybir
from concourse.masks import make_identity

N_CORES = 8
B = 32
BL = B // N_CORES  # samples per core
N = 1024  # NIN == NH
P = 128
C = N // P  # 8 chunks
BETA = 0.95
PI = float(np.pi)
PI3 = PI**3
F32 = mybir.dt.float32
AL = mybir.AluOpType
AF = mybir.ActivationFunctionType

# bench-only knob (test harness): "full" | "dmaonly" | "dmaonly_bl" | "store_bl"
_BENCH_VARIANT = "full"

# Each [4,1024] row-layout tensor lives in its own tile at base partition 0
# (PE matmul requires operand base partitions in {0,32,64} and equal for
# lhsT/rhs; compute engines want operands partition-aligned).
ROW_NAMES = ("inp", "tr1", "mem0", "mem1", "ps0", "ps1",
             "tr1p", "spk0r", "mem0r", "mem1r", "spk1r")


def _emit(nc, tc, ctx, io, n_iters=1, hw_loop=0):
    sp = ctx.enter_context(tc.tile_pool(name="small", bufs=1))
    wraw = ctx.enter_context(tc.tile_pool(name="wraw", bufs=4))
    wtp = ctx.enter_context(tc.tile_pool(name="wtp", bufs=3))
    bcpool = ctx.enter_context(tc.tile_pool(name="bcpool", bufs=8))
    stg = ctx.enter_context(tc.tile_pool(name="stg", bufs=4))
    nwk = ctx.enter_context(tc.tile_pool(name="nwk", bufs=2))
    pp = ctx.enter_context(tc.tile_pool(name="pp", bufs=1, space="PSUM"))
    pcur = ctx.enter_context(tc.tile_pool(name="pcur", bufs=2, space="PSUM"))
    pwt = ctx.enter_context(tc.tile_pool(name="pwt", bufs=3, space="PSUM"))

    # persistent tiles
    rows = {nm: sp.tile([4, N], F32, name=f"r_{nm}") for nm in ROW_NAMES}
    rows["loss"] = sp.tile([4, 10], F32, name="r_loss")
    ident = sp.tile([128, 128], F32)
    inpT = sp.tile([128, BL * C], F32)
    ps05 = sp.tile([128, 5 * C], F32)
    ps15 = sp.tile([128, 5 * C], F32)
    mem0T = sp.tile([128, BL * C], F32)
    mem1T = sp.tile([128, BL * C], F32)
    spk0T = sp.tile([128, BL * C], F32)
    spk1T = sp.tile([128, BL * C], F32)
    m0pT = sp.tile([128, BL * C], F32)
    m1pT = sp.tile([128, BL * C], F32)
    a0T = sp.tile([128, BL * C], F32)
    a1T = sp.tile([128, BL * C], F32)
    cur0T = sp.tile([128, BL * C], F32)
    cur1T = sp.tile([128, BL * C], F32)
    tr0rep = sp.tile([128, N], F32)
    tr1rep = sp.tile([128, N], F32)

    make_identity(nc, ident)
    # quadrant rows 4..31 of the replicated-trace tiles are never written by
    # the per-iteration [4-quadrant, 4-row] DMA; zero them once so
    # stream_shuffle's full-tile read sees initialized data
    nc.gpsimd.memset(tr0rep, 0.0)
    nc.gpsimd.memset(tr1rep, 0.0)

    t = dict(rows=rows, ident=ident, inpT=inpT,
             ps05=ps05, ps15=ps15, mem0T=mem0T, mem1T=mem1T,
             spk0T=spk0T, spk1T=spk1T, m0pT=m0pT, m1pT=m1pT,
             a0T=a0T, a1T=a1T, cur0T=cur0T, cur1T=cur1T,
             tr0rep=tr0rep, tr1rep=tr1rep)
    pools = dict(wraw=wraw, wtp=wtp, bcpool=bcpool, stg=stg, nwk=nwk,
                 pp=pp, pcur=pcur, pwt=pwt)
    if hw_loop:
        # hardware loop for timing: body traced once, looped on device
        with tc.For_i(0, hw_loop, 1, hint_engines=(mybir.EngineType.PE,)):
            _emit_iter(nc, io, t, pools)
    else:
        for _ in range(n_iters):
            _emit_iter(nc, io, t, pools)


def _emit_iter(nc, io, t, pools):
    rows, ident = t["rows"], t["ident"]
    wraw, bcpool, stg, nwk = pools["wraw"], pools["bcpool"], pools["stg"], pools["nwk"]
    wtp = pools["wtp"]
    pp, pcur, pwt = pools["pp"], pools["pcur"], pools["pwt"]

    # ---- input DMAs (W0 row-blocks first: they gate the compute lead-in) ----
    if _BENCH_VARIANT.startswith("dmaonly"):
        # DMA-skeleton bench: loads + garbage dW stores, nothing else
        for o_dW in (io["o_dW0"], io["o_dW1"]):
            for l in range(C):
                wblk = wraw.tile([128, N], F32, tag="wblk", name="wblk")
                nc.sync.dma_start(out=wblk, in_=io["W0"][l * P:(l + 1) * P, :])
                stage = stg.tile([128, BL * N], F32, tag="stage", name="stage")
                nc.gpsimd.memset(stage[:, 0:4], 0.0)
                eng = nc.scalar if l % 2 == 0 else nc.sync
                if _BENCH_VARIANT == "dmaonly":
                    eng.dma_start(
                        out=o_dW[:, l * P:(l + 1) * P, :].rearrange("b p j -> p b j"),
                        in_=stage.rearrange("p (b j) -> p b j", b=BL),
                    )
                else:
                    for b in range(BL):
                        eng = nc.scalar if (l * BL + b) % 2 == 0 else nc.sync
                        eng.dma_start(
                            out=o_dW[b, l * P:(l + 1) * P, :],
                            in_=stage[:, b * N:(b + 1) * N],
                        )
        return
    w0b = []
    for l in range(C):
        wblk = wraw.tile([128, N], F32, tag="wblk")
        eng = nc.sync if l % 2 == 0 else nc.scalar
        eng.dma_start(out=wblk, in_=io["W0"][l * P:(l + 1) * P, :])
        w0b.append(wblk)
    for name in ("inp", "tr1", "mem0", "mem1", "ps0", "ps1"):
        nc.sync.dma_start(out=rows[name], in_=io[name])
    w1b = []
    for l in range(C):
        wblk = wraw.tile([128, N], F32, tag="wblk")
        eng = nc.sync if l % 2 == 0 else nc.scalar
        eng.dma_start(out=wblk, in_=io["W1"][l * P:(l + 1) * P, :])
        w1b.append(wblk)

    # ---- transpose [4,1024] row tensors into T layout [128, C*4] ----
    def pack_T(dst, src):
        ppk = pp.tile([128, BL * C], F32, tag="ppk")
        for c in range(C):
            nc.tensor.transpose(
                ppk[:, c * BL:(c + 1) * BL],
                src[0:4, c * P:(c + 1) * P],
                ident[0:4, 0:4],
            )
        nc.scalar.copy(dst, ppk)

    def pack_T5(dst, src):
        # like pack_T but into [128, C*5] with column 4 of each group = 1.0
        ppk = pp.tile([128, BL * C], F32, tag="ppk")
        for c in range(C):
            nc.tensor.transpose(
                ppk[:, c * BL:(c + 1) * BL],
                src[0:4, c * P:(c + 1) * P],
                ident[0:4, 0:4],
            )
        dv = dst.rearrange("p (c f) -> p c f", f=5)
        nc.scalar.copy(dv[:, :, 0:4], ppk.rearrange("p (c b) -> p c b", b=BL))
        nc.gpsimd.memset(dv[:, :, 4:5], 1.0)

    pack_T(t["inpT"], rows["inp"])
    pack_T(t["mem0T"], rows["mem0"])
    pack_T(t["mem1T"], rows["mem1"])
    pack_T5(t["ps05"], rows["ps0"])
    pack_T5(t["ps15"], rows["ps1"])

    # ---- trace update + broadcast rows to 128 partitions ----
    # the [4,N] trace rows go to a DRAM scratch, come back replicated into
    # rows 0-3 of each 32-partition quadrant (4-quadrant strided read), and a
    # DVE stream_shuffle (mask=[b]*32, per-quadrant) broadcasts row b to all
    # 128 partitions -- no TensorE/PSUM involvement.
    def bcast4(src_rows, scratch, rep):
        if src_rows is not None:
            nc.sync.dma_start(out=scratch, in_=src_rows)
        for q in range(4):
            nc.scalar.dma_start(out=rep[q * 32:q * 32 + 4, :], in_=scratch)
        out = []
        for b in range(BL):
            bc = bcpool.tile([128, N], F32, tag="bc")
            nc.vector.stream_shuffle(bc, rep, mask=[b] * 32)
            out.append(bc)
        return out

    # io["tr0"] already holds tr0' = beta*tr0 + inp (host-folded, pure-input)
    bc0 = bcast4(None, io["tr0"], t["tr0rep"])

    # ---- one layer ----
    # Phase A (per chunk l): W row-block transposes + 8 cur matmuls + PSUM->
    # SBUF copy.  Phase B (once): one batched DVE neuron chain on the whole
    # [128, 32] state.  Phase C: loss matmuls + 64 fills / 64 stores stream.
    # Batching phase B removes ~100 cross-engine handoffs from the critical
    # path; the stores overlap the next layer's phase A.
    def layer(wblocks, rhsT, memT, ps5, spkT, mpT, aT, bc, o_dW, curT):
        lossp = pp.tile([4, 5], F32, tag="ploss")
        for half in range(2):
            _layer_half(wblocks, rhsT, memT, ps5, spkT, mpT, aT, bc, o_dW,
                        curT, lossp, half)
        return lossp

    def _layer_half(wblocks, rhsT, memT, ps5, spkT, mpT, aT, bc, o_dW,
                    curT, lossp, half):
        hc = C // 2
        for l in range(half * hc, (half + 1) * hc):
            # transpose W row-block l -> wtl[:, c, :] = W[l-block, c-block].T
            # (these 8 tiles are exactly the lhsT set for cur chunk l)
            wtl = wtp.tile([128, C * P], F32, tag="wtl")
            wtlv = wtl.rearrange("p (c h) -> p c h", h=P)
            for g in range(2):
                pw = pwt.tile([128, 512], F32, tag="pwt")
                for q in range(4):
                    c = g * 4 + q
                    nc.tensor.transpose(
                        pw[:, q * P:(q + 1) * P],
                        wblocks[l][:, c * P:(c + 1) * P], ident,
                    )
                cp = nc.vector.tensor_copy if g == 0 else nc.scalar.copy
                cp(
                    wtlv[:, g * 4:(g + 1) * 4, :],
                    pw.rearrange("p (q h) -> p q h", h=P),
                )
            # cur.T chunk l accumulates over the 8 contraction chunks
            cps = pcur.tile([128, BL], F32, tag="cur")
            for c in range(C):
                nc.tensor.matmul(
                    cps, lhsT=wtlv[:, c, :],
                    rhs=rhsT[:, c * BL:(c + 1) * BL],
                    start=(c == 0), stop=(c == C - 1),
                )
            sl = slice(l * BL, (l + 1) * BL)
            nc.scalar.copy(curT[:, sl], cps)
        # batched neuron state + surrogate on this half: [128, hc*BL]
        hs = slice(half * hc * BL, (half + 1) * hc * BL)
        memp = nwk.tile([128, BL * hc], F32, tag="memp")
        nc.vector.scalar_tensor_tensor(
            out=memp, in0=memT[:, hs], scalar=BETA, in1=curT[:, hs],
            op0=AL.mult, op1=AL.add,
        )
        nc.vector.tensor_scalar(spkT[:, hs], memp, 1.0, None, op0=AL.is_gt)
        nc.vector.tensor_sub(mpT[:, hs], memp, spkT[:, hs])
        xm1 = nwk.tile([128, BL * hc], F32, tag="xm1")
        nc.vector.tensor_scalar(xm1, mpT[:, hs], -1.0, None, op0=AL.add)
        sq = nwk.tile([128, BL * hc], F32, tag="sq")
        nc.vector.tensor_mul(sq, xm1, xm1)
        den = nwk.tile([128, BL * hc], F32, tag="den")
        nc.vector.tensor_scalar(den, sq, PI3, PI, op0=AL.mult, op1=AL.add)
        rec = nwk.tile([128, BL * hc], F32, tag="rec")
        nc.vector.reciprocal(rec, den)
        ps5v = ps5.rearrange("p (c f) -> p c f", f=5)
        nc.vector.tensor_mul(
            aT.rearrange("p (c b) -> p c b", b=BL)[:, half * hc:(half + 1) * hc, :],
            ps5v[:, half * hc:(half + 1) * hc, 0:4],
            rec.rearrange("p (c b) -> p c b", b=BL),
        )
        # loss reduction: [4,5] += spk_chunk.T @ [prev' | 1] per chunk
        for l in range(half * hc, (half + 1) * hc):
            nc.tensor.matmul(
                lossp, lhsT=spkT[:, l * BL:(l + 1) * BL],
                rhs=ps5[:, l * 5:(l + 1) * 5],
                start=(l == 0), stop=(l == C - 1),
            )
        # dW: stage[p, b*N + j] = a[l*128+p, b] * tr'[b, j]; contiguous
        # per-sample 512 KB stores alternating the two HWDGE rings
        for l in range(half * hc, (half + 1) * hc):
            stage = stg.tile([128, BL * N], F32, tag="stage")
            for b in range(BL):
                scal = aT[:, l * BL + b:l * BL + b + 1]
                dst = stage[:, b * N:(b + 1) * N]
                if b == 3:
                    nc.scalar.activation(dst, bc[b], AF.Copy, bias=0.0, scale=scal)
                else:
                    nc.vector.tensor_scalar_mul(dst, bc[b], scal)
            for b in range(BL):
                eng = nc.scalar if (l * BL + b) % 2 == 0 else nc.sync
                eng.dma_start(
                    out=o_dW[b, l * P:(l + 1) * P, :],
                    in_=stage[:, b * N:(b + 1) * N],
                )

    lossp0 = layer(w0b, t["inpT"], t["mem0T"], t["ps05"],
                   t["spk0T"], t["m0pT"], t["a0T"], bc0, io["o_dW0"], t["cur0T"])
    nc.vector.tensor_copy(rows["loss"][0:4, 0:5], lossp0)

    # ---- spk0 back to row layout, layer-1 trace + broadcast ----
    def to_rows(srcT, dst):
        for g in range(2):
            pr = pp.tile([4, 512], F32, tag="prow")
            for q in range(4):
                c = g * 4 + q
                nc.tensor.transpose(
                    pr[0:4, q * P:(q + 1) * P],
                    srcT[:, c * BL:(c + 1) * BL], ident,
                )
            nc.vector.tensor_copy(
                dst[0:4, g * 512:(g + 1) * 512], pr[0:4, :]
            )

    to_rows(t["spk0T"], rows["spk0r"])
    to_rows(t["m0pT"], rows["mem0r"])
    nc.gpsimd.dma_start(out=io["o_mem0"], in_=rows["mem0r"])
    nc.vector.scalar_tensor_tensor(
        out=rows["tr1p"], in0=rows["tr1"],
        scalar=BETA, in1=rows["spk0r"], op0=AL.mult, op1=AL.add,
    )
    bc1 = bcast4(rows["tr1p"], io["tr1p_d"], t["tr1rep"])

    lossp1 = layer(w1b, t["spk0T"], t["mem1T"], t["ps15"],
                   t["spk1T"], t["m1pT"], t["a1T"], bc1, io["o_dW1"], t["cur1T"])
    nc.vector.tensor_copy(rows["loss"][0:4, 5:10], lossp1)

    # ---- outputs (mem0 emitted right after layer 0 via program order) ----
    to_rows(t["m1pT"], rows["mem1r"])
    to_rows(t["spk1T"], rows["spk1r"])
    nc.gpsimd.dma_start(out=io["o_mem1"], in_=rows["mem1r"])
    nc.gpsimd.dma_start(out=io["o_spk1"], in_=rows["spk1r"])
    nc.gpsimd.dma_start(
        out=io["o_loss"].rearrange("k b f -> b k f"),
        in_=rows["loss"].rearrange("b (k f) -> b k f", k=2),
    )


def build(n_iters=1, hw_loop=0):
    nc = bacc.Bacc("TRN2", debug=False, num_devices=N_CORES)
    io = {}
    for name, shape in (
        ("inp", [BL, N]), ("W0", [N, N]), ("W1", [N, N]),
        ("mem0", [BL, N]), ("mem1", [BL, N]),
        ("ps0", [BL, N]), ("ps1", [BL, N]),
        ("tr0", [BL, N]), ("tr1", [BL, N]),
    ):
        io[name] = nc.dram_tensor(name, shape, F32, kind="ExternalInput").ap()
    for name, shape in (
        ("o_spk1", [BL, N]), ("o_mem0", [BL, N]), ("o_mem1", [BL, N]),
        ("o_loss", [2, BL, 5]),
        ("o_dW0", [BL, N, N]), ("o_dW1", [BL, N, N]),
    ):
        io[name] = nc.dram_tensor(name, shape, F32, kind="ExternalOutput").ap()
    for name in ("tr0p_d", "tr1p_d"):
        io[name] = nc.dram_tensor(name, [BL, N], F32, kind="Internal").ap()
    with tile.TileContext(nc) as tc:
        with ExitStack() as ctx:
            _emit(nc, tc, ctx, io, n_iters=n_iters, hw_loop=hw_loop)
    nc.compile()
    return nc


class Runner:
    """Compile once, execute many times (replicates bass2jax.run_bass_via_pjrt
    multi-core path but keeps the jitted callable across calls)."""

    def __init__(self, nc, n_cores=N_CORES):
        import jax
        from concourse import bass2jax

        bass2jax.install_neuronx_cc_hook()
        self.nc = nc
        self.n_cores = n_cores
        self.jax = jax
        partition_name = (
            nc.partition_id_tensor.name if nc.partition_id_tensor else None
        )
        in_names, out_names, out_avals = [], [], []
        self.zero_shapes = []
        for alloc in nc.m.functions[0].allocations:
            if not isinstance(alloc, mybir.MemoryLocationSet):
                continue
            name = alloc.memorylocations[0].name
            if alloc.kind == "ExternalInput":
                if name != partition_name:
                    in_names.append(name)
            elif alloc.kind == "ExternalOutput":
                out_names.append(name)
                shape = tuple(alloc.tensor_shape)
                dtype = mybir.dt.np(alloc.dtype)
                out_avals.append(jax.core.ShapedArray(shape, dtype))
                self.zero_shapes.append((shape, dtype))
        self.n_params = len(in_names)
        self.out_names = list(out_names)
        all_in = in_names + out_names
        if partition_name is not None:
            all_in.append(partition_name)
        self.in_names = all_in
        self.out_avals = out_avals

        from jax.sharding import Mesh, PartitionSpec, NamedSharding

        try:
            from jax.experimental.shard_map import shard_map
        except ImportError:
            from jax.shard_map import shard_map

        def _body(*args):
            operands = list(args)
            if partition_name is not None:
                operands.append(bass2jax.partition_id_tensor())
            outs = bass2jax._bass_exec_p.bind(
                *operands,
                out_avals=tuple(out_avals),
                in_names=tuple(all_in),
                out_names=tuple(out_names),
                lowering_input_output_aliases=(),
                sim_require_finite=True,
                sim_require_nnan=True,
                nc=nc,
            )
            return tuple(outs)

        devices = jax.devices()[:n_cores]
        assert len(devices) == n_cores
        self.mesh = Mesh(np.asarray(devices), ("core",))
        n_outs = len(out_names)
        donate = tuple(range(self.n_params, self.n_params + n_outs))
        in_specs = (PartitionSpec("core"),) * (self.n_params + n_outs)
        out_specs = (PartitionSpec("core"),) * n_outs
        self.sharding = NamedSharding(self.mesh, PartitionSpec("core"))
        self.fn = jax.jit(
            shard_map(
                _body, mesh=self.mesh, in_specs=in_specs,
                out_specs=out_specs, check_rep=False,
            ),
            donate_argnums=donate, keep_unused=True,
        )
        self._zeros_fn = jax.jit(
            lambda: tuple(
                self.jax.numpy.zeros((n_cores * s[0], *s[1:]), d)
                for s, d in self.zero_shapes
            ),
            out_shardings=tuple([self.sharding] * n_outs),
        )

    def put_inputs(self, in_maps):
        """Concat per-core input dicts and device_put once."""
        concat = [
            np.concatenate(
                [np.asarray(m[name]) for m in in_maps], axis=0
            )
            for name in self.in_names[: self.n_params]
        ]
        return [
            self.jax.device_put(a, self.sharding) for a in concat
        ]

    def run(self, dev_inputs):
        zeros = self._zeros_fn()
        outs = self.fn(*dev_inputs, *zeros)
        self.jax.block_until_ready(outs)
        return outs

    def results(self, outs):
        res = []
        for c in range(self.n_cores):
            d = {}
            for i, name in enumerate(self.out_names):
                full = np.asarray(outs[i])
                d[name] = full.reshape(
                    self.n_cores, *self.out_avals[i].shape
                )[c]
            res.append(d)
        return res


_RUNNER = None


def _get_runner():
    global _RUNNER
    if _RUNNER is None:
        _RUNNER = Runner(build(n_iters=1))
    return _RUNNER


def kernel(inp, W0, W1, mem0, mem1, prev_spk0, prev_spk1,
           inp_trace0, inp_trace1, bf):
    f = lambda x: np.asarray(x, dtype=np.float32)
    inp, W0, W1 = f(inp), f(W0), f(W1)
    mem0, mem1 = f(mem0), f(mem1)
    ps0 = f(prev_spk0) * float(bf)  # bf folded into prev_spk (linear)
    ps1 = f(prev_spk1) * float(bf)
    # layer-0 trace update is pure-input: fold it on the host like bf
    tr0 = BETA * f(inp_trace0) + f(inp)
    tr1 = f(inp_trace1)

    runner = _get_runner()
    in_maps = []
    for c in range(N_CORES):
        s = slice(c * BL, (c + 1) * BL)
        in_maps.append({
            "inp": inp[s], "W0": W0, "W1": W1,
            "mem0": mem0[s], "mem1": mem1[s],
            "ps0": ps0[s], "ps1": ps1[s],
            "tr0": tr0[s], "tr1": tr1[s],
        })
    outs = runner.run(runner.put_inputs(in_maps))
    res = runner.results(outs)

    spk1 = np.concatenate([r["o_spk1"] for r in res], axis=0)
    mem0o = np.concatenate([r["o_mem0"] for r in res], axis=0)
    mem1o = np.concatenate([r["o_mem1"] for r in res], axis=0)
    dW0 = np.concatenate([r["o_dW0"] for r in res], axis=0)
    dW1 = np.concatenate([r["o_dW1"] for r in res], axis=0)

    # losses: device gives, per layer, s1[b,b'] = sum_o spk[o,b] prev'[o,b']
    # (cols 0..3) and s2[b] = sum_o spk[o,b] (col 4).
    # loss_b = -(s1[b,b] - mean(prev'[b]) * s2[b]);  bf already in prev'.
    l0, l1 = [], []
    for c, r in enumerate(res):
        s = slice(c * BL, (c + 1) * BL)
        mu0 = ps0[s].mean(axis=1)
        mu1 = ps1[s].mean(axis=1)
        lr = r["o_loss"]
        for b in range(BL):
            l0.append(-(lr[0, b, b] - mu0[b] * lr[0, b, 4]))
            l1.append(-(lr[1, b, b] - mu1[b] * lr[1, b, 4]))
    losses = np.array([np.mean(l0), np.mean(l1)], dtype=np.float32)

    return spk1, mem0o, mem1o, losses, dW0, dW1
